# revision 62
# baseline (speedup 1.0000x reference)
"""Trainium2 Bass kernel for nn_CPLinear (CP-decomposed QKV projection with RoPE).

Computes, for x:(2,4096,2048) and CP-factor weights:
    A_t = x @ W_A_t  (per-token head coefficients),  B_t = x @ W_B_t (shared bases)
    q = einsum('bshr,bsrd->bshd', A_q, rope(B_q)) / 12
    k = A_k * rope(B_k)   (rank-1)
    v = A_v * B_v         (rank-1)

Strategy (8 cores, data-parallel over the 8192 tokens, 1024 tokens/core);
HW exec ~159us mean (158-161 across runs) vs the 194us naive baseline:
  - All 6 projections fused into one [2048 x 2016] bf16 matmul; each k-chunk
    runs 4 matmuls off one stationary load (LDWEIGHTS fully amortized).
  - W streams in k-chunks, ONE whole-chunk DMA per chunk alternating the two
    HWDGE queues (halves trigger count + shared-HWDGE generation time), with
    the first two tiles' x sliver appended to W host-side; those two tiles
    run k-major, paced by chunk arrival. Remaining x arrives per-tile behind
    the stream, pre-tiled host-side into contiguous-run DMAs.
  - PE pre-warm: ~30 dummy matmuls into a scratch PSUM bank while the first
    chunk streams, so real matmuls start with the p-state ramp (0.65 ->
    1.2 -> 2.4GHz over ~3us of continuous work) already absorbed.
  - PSUM: psa[512]x1 + psb[1536]x2 + psq[512]x1 = 8 banks; stage-1 evicts
    psa(1) (parked in the psq bank) first so the BD(0) weave isn't blocked.
  - The rank-12 q contraction runs as block-diagonal matmuls (8 tokens per
    matmul, K=96, operands built via a DRAM bounce + 3-dim scatter APs;
    partition-crossing scatters cannot go SBUF->SBUF: the verifier requires
    partition-dim-first APs on the SBUF side). BD(p-2) groups 1-3 weave into
    proj(p)'s k-chunks (psq needs >=1.3us between groups for the ACT evict
    round-trip); BD(p-1) group 0 runs right after post_b(p).
  - Last tile: A-block columns first (short A'/k/v chains start early), then
    B_q; BD(5) groups weave into the A-pass, BD(6) into the B-pass, so only
    BD(7) remains after the bounce round-trip, evictions ping-ponging the
    psq/psa banks. Tail tiles 6,7 invert the bounce (8 scatter WRITES into a
    BD-layout DRAM buffer + one contiguous 4KB-row read) so bdr lands in one
    semaphore instead of eight.
  - kv ships as one combined [128,4096] DMA per tile; cos/sin tables arrive
    as a single merged SWDGE DMA; q is written in raw block-diagonal layout
    and untangled on the host.
"""

import sys

for _p in ("/opt/trn_rl_repo",):
    if _p not in sys.path:
        sys.path.insert(0, _p)

import numpy as np
import ml_dtypes

BF16 = ml_dtypes.bfloat16

SH = 1024          # tokens per core
H = 2048           # hidden
KT = H // 128      # 16 k-tiles
NT = SH // 128     # 8 token tiles per core
NOUT = 2016        # fused projection output width
WX = NOUT + 256    # W chunk + stage-1 x sliver, merged for one-DMA-per-queue
WSPL = 1136        # k-chunk DMA column split between the two HWDGE queues
NH, HD, RQ = 16, 128, 12
N_WARM = 40        # PE pre-warm dummy matmuls (128 cols each)

_CACHE = {}


def make_nc():
    import concourse.bacc as bacc
    from concourse import mybir

    dt = mybir.dt

    nc = bacc.Bacc(
        "TRN2",
        target_bir_lowering=False,
        debug=False,
        enable_asserts=False,
        num_devices=8,
    )

    x_d = nc.dram_tensor("x", (NT - 2, 128, KT * 128), dt.bfloat16, kind="ExternalInput")  # pre-tiled host-side
    w_d = nc.dram_tensor("w", (KT, 128, WX), dt.bfloat16, kind="ExternalInput")
    cs_d = nc.dram_tensor("cs", (SH, 128), dt.bfloat16, kind="ExternalInput")
    q_d = nc.dram_tensor("q", (NT, 128, NH * HD), dt.bfloat16, kind="ExternalOutput")
    kv_d = nc.dram_tensor("kv", (SH, 2 * NH * HD), dt.bfloat16, kind="ExternalOutput")
    return nc, (x_d, w_d, cs_d, q_d, kv_d)


def build_body(nc, tc, tensors):
    from contextlib import ExitStack

    from concourse import mybir

    dt = mybir.dt
    x_d, w_d, cs_d, q_d, kv_d = tensors

    with ExitStack() as ctx:
        P = ctx.enter_context
        const_pool = P(tc.tile_pool(name="const", bufs=1))
        w_sb = const_pool.tile([128, KT * WX], dt.bfloat16, tag="w_sb")
        xT = const_pool.tile([128, (NT - 2) * KT * 128], dt.bfloat16, tag="xT")
        cs_sb = const_pool.tile([128, NT * 128], dt.bfloat16, tag="cs_sb")
        warm = const_pool.tile([128, 128], dt.bfloat16, tag="warm")

        lhs_bufs = [
            const_pool.tile([128, 2048], dt.bfloat16, tag=f"lhs{i}", name=f"lhs{i}")
            for i in range(4)
        ]
        bdr_bufs = [
            const_pool.tile([128, 2048], dt.bfloat16, tag=f"bdr{i}", name=f"bdr{i}")
            for i in range(4)
        ]

        # ---- startup DMAs: k-chunked, split across both HWDGE queues ----
        # cos||sin for all 8 tiles in one SWDGE DMA (per tile p: cols
        # [p*128, p*128+64) = cos, [p*128+64, (p+1)*128) = sin)
        nc.gpsimd.dma_start(
            out=cs_sb[:].rearrange("p (t n) -> p t n", t=NT),
            in_=cs_d[:].rearrange("(t p) n -> p t n", p=128),
        )
        for tl in lhs_bufs:
            nc.gpsimd.memset(tl[:], 0.0)
        # stage-1 stream: W with tiles 0-1's x sliver appended host-side.
        # One whole-chunk DMA per k-chunk, alternating queues: halves the
        # trigger count and the shared-HWDGE generation serialization, so
        # early chunks land sooner.
        # chunks 0-2 gate the PE start: split each across both queues so
        # their transfers halve; later chunks ship whole, alternating queues
        for kk in range(3):
            nc.scalar.dma_start(
                out=w_sb[:, kk * WX : kk * WX + WSPL],
                in_=w_d[kk][:, 0:WSPL],
            )
            nc.sync.dma_start(
                out=w_sb[:, kk * WX + WSPL : (kk + 1) * WX],
                in_=w_d[kk][:, WSPL:WX],
            )
        for kk in range(3, KT):
            eng = nc.scalar if kk % 2 == 0 else nc.sync
            eng.dma_start(
                out=w_sb[:, kk * WX : (kk + 1) * WX],
                in_=w_d[kk][:, 0:WX],
            )
        # the remaining x tokens stream per-tile behind the stage-1 stream,
        # pre-tiled host-side so each per-tile load is one contiguous-run DMA
        for p in range(2, NT):
            eng = nc.sync if p % 2 == 0 else nc.scalar
            eng.dma_start(
                out=xT[:, (p - 2) * KT * 128 : (p - 1) * KT * 128],
                in_=x_d[p - 2],
            )



        psa_pool = P(tc.tile_pool(name="psa", bufs=1, space="PSUM"))
        psb_pool = P(tc.tile_pool(name="psb", bufs=2, space="PSUM"))
        psq_pool = P(tc.tile_pool(name="psq", bufs=1, space="PSUM"))
        small_pool = P(tc.tile_pool(name="small", bufs=3))
        bq_pool = P(tc.tile_pool(name="bq", bufs=2))
        bqr_pool = P(tc.tile_pool(name="bqr", bufs=2))
        rope_pool = P(tc.tile_pool(name="rope", bufs=3))
        kv_pool = P(tc.tile_pool(name="kvouts", bufs=4))
        q_pool = P(tc.tile_pool(name="qouts", bufs=4))
        scr_pool = P(tc.tile_pool(name="scr", bufs=3, space="DRAM"))

        # ---- PE pre-warm: dummy matmuls while the first W chunk streams ----
        # The PE p-state ramps 0.65->1.2->2.4GHz over ~3us of continuous
        # work; idle-starting on chunk 0 would run the first ~3us of real
        # matmuls at half speed.  Warm the array on throwaway data so the
        # first real matmul issues into a full-speed pipeline.
        nc.vector.memset(warm[:], 0.0)
        ps_warm = psq_pool.tile([128, 128], dt.float32, tag="qp", name="warm")
        for _ in range(N_WARM):
            nc.tensor.matmul(ps_warm[:], warm[:], warm[:], start=True, stop=True)

        def proj_chunk(p, ps_a, ps_b, kk, parts="ab"):
            t0 = p * 128
            if p < 2:
                lh = w_sb[:, kk * WX + NOUT + t0 : kk * WX + NOUT + t0 + 128]
            else:
                lh = xT[:, ((p - 2) * KT + kk) * 128 : ((p - 2) * KT + kk) * 128 + 128]
            wb = kk * WX
            st = kk == 0
            sp = kk == KT - 1
            if "a" in parts:
                nc.tensor.matmul(
                    ps_a[:, 0:480], lh, w_sb[:, wb : wb + 480], start=st,
                    stop=sp,
                )
            if "b" in parts:
                for c in range(3):
                    nc.tensor.matmul(
                        ps_b[:, c * 512 : (c + 1) * 512],
                        lh,
                        w_sb[:, wb + 480 + c * 512 : wb + 480 + (c + 1) * 512],
                        start=st,
                        stop=sp,
                    )

        state = {}

        def ensure_state(p):
            if p not in state:
                scr = scr_pool.tile([128, 1728], dt.bfloat16, tag="scr",
                                    name=f"scr{p}")
                state[p] = {"scr": scr}
            return state[p]

        def post_a(p, ps_a):
            """psa eviction + A' bounce + ropeK + k/v for proj tile p."""
            t0 = p * 128
            st = ensure_state(p)
            scr = st["scr"]
            smalls = small_pool.tile([128, 480], dt.bfloat16, tag="smalls")
            bkr = small_pool.tile([128, 128], dt.bfloat16, tag="bkr")
            tka = small_pool.tile([128, 64], dt.bfloat16, tag="tka")
            tkb = small_pool.tile([128, 64], dt.bfloat16, tag="tkb")
            nc.scalar.copy(smalls[:], ps_a[:, 0:480])
            # A' -> scratch (read back by l_v)
            nc.gpsimd.dma_start(out=scr[:, 1536:1728], in_=smalls[:, 0:192])
            st.update({"smalls": smalls, "bkr": bkr, "tka": tka, "tkb": tkb})

        def post_kv(p, tail=False):
            """ropeK + k/v rank-1 broadcasts + outputs for proj tile p.

            tail=True moves the v broadcast to gpsimd so the final tile's
            k and v expansions run on two engines in parallel."""
            t0 = p * 128
            st = state[p]
            smalls, bkr, tka, tkb = (st["smalls"], st["bkr"], st["tka"],
                                     st["tkb"])
            cos_k = cs_sb[:, p * 128 : p * 128 + 64]
            sin_k = cs_sb[:, p * 128 + 64 : (p + 1) * 128]
            bkv = smalls[:, 224:352].rearrange("p (two d) -> p two d", two=2)
            bkrv = bkr[:].rearrange("p (two d) -> p two d", two=2)
            nc.vector.tensor_mul(tka[:], bkv[:, 0], cos_k)
            nc.vector.tensor_mul(tkb[:], bkv[:, 1], sin_k)
            nc.vector.tensor_sub(bkrv[:, 0], tka[:], tkb[:])
            nc.vector.tensor_mul(tka[:], bkv[:, 1], cos_k)
            nc.vector.tensor_mul(tkb[:], bkv[:, 0], sin_k)
            nc.vector.tensor_add(bkrv[:, 1], tka[:], tkb[:])
            kvsb = kv_pool.tile([128, 4096], dt.bfloat16, tag="ksb")
            v_eng = nc.gpsimd if tail else nc.vector
            v_eng.tensor_mul(
                kvsb[:, 2048:4096].rearrange("p (h d) -> p h d", h=NH),
                smalls[:, 352:480].unsqueeze(1).broadcast_to([128, NH, 128]),
                smalls[:, 208:224].unsqueeze(2).broadcast_to([128, NH, 128]),
            )
            nc.vector.tensor_mul(
                kvsb[:, 0:2048].rearrange("p (h d) -> p h d", h=NH),
                bkr[:].unsqueeze(1).broadcast_to([128, NH, 128]),
                smalls[:, 192:208].unsqueeze(2).broadcast_to([128, NH, 128]),
            )
            nc.sync.dma_start(out=kv_d[t0 : t0 + 128, :], in_=kvsb[:])

        def post_b(p, ps_b, fast_tail=False, do_lv=True):
            """psb eviction, rope on B_q, bounce + scatter reads for tile p."""
            st = ensure_state(p)
            scr = st["scr"]
            bq = bq_pool.tile([128, 1536], dt.bfloat16, tag="bq")
            # last tile: evict on DVE so the ACT queue stays clear for the
            # woven BD-group evictions that pace the tail
            if fast_tail:
                nc.vector.tensor_copy(bq[:], ps_b[:])
            else:
                nc.scalar.copy(bq[:], ps_b[:])
            bqr = bqr_pool.tile([128, 1536], dt.bfloat16, tag="bqr")
            ta = rope_pool.tile([128, 768], dt.bfloat16, tag="ta")
            tb = rope_pool.tile([128, 768], dt.bfloat16, tag="tb")
            cosr = (
                cs_sb[:, p * 128 : p * 128 + 64]
                .unsqueeze(1)
                .broadcast_to([128, RQ, 64])
            )
            sinr = (
                cs_sb[:, p * 128 + 64 : (p + 1) * 128]
                .unsqueeze(1)
                .broadcast_to([128, RQ, 64])
            )
            sv = bq[:].rearrange("p (r two d) -> p r two d", r=RQ, two=2)
            dv = bqr[:].rearrange("p (r two d) -> p r two d", r=RQ, two=2)
            tav = ta[:].rearrange("p (r d) -> p r d", r=RQ)
            tbv = tb[:].rearrange("p (r d) -> p r d", r=RQ)
            p_lo = sv[:, :, 0]
            p_hi = sv[:, :, 1]
            nc.vector.tensor_mul(tav, p_lo, cosr)
            nc.vector.tensor_mul(tbv, p_hi, sinr)
            nc.vector.tensor_sub(dv[:, :, 0], tav, tbv)
            nc.vector.tensor_mul(tav, p_hi, cosr)
            nc.vector.tensor_mul(tbv, p_lo, sinr)
            nc.vector.tensor_add(dv[:, :, 1], tav, tbv)

            bdr = bdr_bufs[p % 4]
            if p >= NT - 2:
                # tail tiles: invert the bounce — 8 scatter WRITES land the
                # DRAM buffer already in BD layout, then one contiguous
                # 4KB-row read delivers all of bdr at once, so the final BD
                # groups are not paced by 8 trickling scatter-read sems.
                # The writes go out BEFORE the lv_reads so the gpsimd queue
                # doesn't head-of-line block the B-side chain.
                scr2 = scr_pool.tile([96, 2048], dt.bfloat16, tag="scr2",
                                     name=f"scr2_{p}")
                w_v = bqr[:].rearrange("(g t) (r d) -> t g r d", t=8, r=RQ)
                s_v = scr2[:].rearrange("(t r) (g d) -> t g r d", t=8, g=16)
                for t in range(8):
                    eng = (nc.sync, nc.scalar, nc.gpsimd)[t % 3]
                    eng.dma_start(out=s_v[t], in_=w_v[t])
                nc.sync.dma_start(out=bdr[0:96, :], in_=scr2[:])
                st["bdr"] = bdr
                if do_lv:
                    lv_reads(p)
                return
            # bounce roped B_q, then scatter-read the block-diagonal operands
            nc.scalar.dma_start(out=scr[:, 0:1536], in_=bqr[:])
            sb_v = scr[:, 0:1536].rearrange("(g t) (r d) -> t r g d", t=8, r=RQ)
            d_v = bdr[0:96, :].rearrange("(t r) (g d) -> t r g d", t=8, g=16)
            for t in range(8):
                eng = (nc.sync, nc.scalar, nc.gpsimd)[t % 3]
                eng.dma_start(out=d_v[t], in_=sb_v[t])
            st["bdr"] = bdr
            if do_lv:
                lv_reads(p)

        def lv_reads(p, spread=False):
            scr = state[p]["scr"]
            lhs = lhs_bufs[p % 4]
            sa_v = scr[:, 1536:1728].rearrange(
                "(g t) (r h) -> t r g h", t=8, r=RQ
            )
            l_v = lhs[0:96, :].rearrange("(t r) (g c) -> t r g c", t=8, g=16)
            for t in range(8):
                if spread:
                    eng = (nc.scalar, nc.gpsimd)[t % 2]
                else:
                    eng = nc.gpsimd
                eng.dma_start(
                    out=l_v[t][:, :, t * 16 : (t + 1) * 16], in_=sa_v[t]
                )
            state[p]["lhs"] = lhs

        def bd_group(p, gq, pool=None, tag="psa"):
            """one PSUM-bank group (4 block-diagonal matmuls) of tile p's q."""
            st = state[p]
            if gq == 0:
                st["qsb"] = q_pool.tile([128, 2048], dt.bfloat16, tag="qsb",
                                        name=f"qsb{p}")
            lhs, bdr, qsb = st["lhs"], st["bdr"], st["qsb"]
            if pool is None:
                qp = psq_pool.tile([128, 512], dt.float32, tag="qp",
                                   name=f"qp{p}_{gq}")
            else:
                qp = pool.tile([128, 512], dt.float32, tag=tag,
                               name=f"qp{p}_{gq}")
            for j4 in range(4):
                g = gq * 4 + j4
                nc.tensor.matmul(
                    qp[:, j4 * 128 : (j4 + 1) * 128],
                    lhs[0:96, g * 128 : (g + 1) * 128],
                    bdr[0:96, g * 128 : (g + 1) * 128],
                    start=True,
                    stop=True,
                )
            nc.scalar.copy(qsb[:, gq * 512 : (gq + 1) * 512], qp[:])
            if p == NT - 1:
                # last tile: ship each PSUM-group slice as soon as it evicts
                nc.sync.dma_start(
                    out=q_d[p][:, gq * 512 : (gq + 1) * 512],
                    in_=qsb[:, gq * 512 : (gq + 1) * 512],
                )
            elif gq == 3:
                nc.scalar.dma_start(out=q_d[p], in_=qsb[:])

        # ================= schedule =================
        # stage 1: proj tiles 0,1 k-major, paced by the chunked W/x DMAs.
        # tile 1's A-block accumulates in the (otherwise idle) psq pool.
        ps_a0 = psa_pool.tile([128, 512], dt.float32, tag="psa", name="psa0")
        ps_b0 = psb_pool.tile([128, 1536], dt.float32, tag="psb", name="psb0")
        ps_a1 = psq_pool.tile([128, 512], dt.float32, tag="qp", name="psa1")
        ps_b1 = psb_pool.tile([128, 1536], dt.float32, tag="psb", name="psb1")
        for kk in range(KT):
            proj_chunk(0, ps_a0, ps_b0, kk)
            proj_chunk(1, ps_a1, ps_b1, kk)
        post_a(0, ps_a0)
        post_a(1, ps_a1)  # frees the psq bank early for the BD(0) weave
        post_b(0, ps_b0)
        post_b(1, ps_b1)
        post_kv(0)
        post_kv(1)

        # stage 2: tiles 2-6 tile-major with the BD contraction woven in.
        # proj(3) carries the BD(0)/BD(1) catch-up; from then on BD(p-1)
        # starts at post(p) and BD(p-2) groups 1-3 run inside proj(p).
        weave = {
            3: [(0, 0, 1), (0, 1, 3), (0, 2, 5), (0, 3, 7),
                (1, 0, 9), (1, 1, 11), (1, 2, 13), (1, 3, 15)],
        }
        for p in range(4, NT):
            weave[p] = [(p - 2, 1, 3), (p - 2, 2, 7), (p - 2, 3, 11)]
        for p in range(2, NT - 1):
            ps_a = psa_pool.tile([128, 512], dt.float32, tag="psa",
                                 name=f"psa{p}")
            ps_b = psb_pool.tile([128, 1536], dt.float32, tag="psb",
                                 name=f"psb{p}")
            slots = {kk: (bp, gq) for (bp, gq, kk) in weave.get(p, [])}
            for kk in range(KT):
                proj_chunk(p, ps_a, ps_b, kk)
                if kk in slots:
                    bd_group(*slots[kk])
            post_a(p, ps_a)
            post_b(p, ps_b)
            if p >= 3:
                bd_group(p - 1, 0)
            post_kv(p)
        # last tile: A-block columns first so the A'/k/v chains run during
        # the B_q pass, which in turn ends early enough that the bounce
        # round-trip hides under the reserved BD groups
        p = NT - 1
        ps_a = psa_pool.tile([128, 512], dt.float32, tag="psa", name="psa7")
        ps_b = psb_pool.tile([128, 1536], dt.float32, tag="psb", name="psb7")
        a_slots = {4: (NT - 3, 1), 12: (NT - 3, 2)}
        for kk in range(KT):
            proj_chunk(p, ps_a, ps_b, kk, parts="a")
            if kk in a_slots:
                bd_group(*a_slots[kk])
        post_a(p, ps_a)
        lv_reads(p)
        post_kv(p)
        # BD(5)/BD(6) finish inside the B_q pass, so their evictions and q
        # DMAs land before the tail crunch; only BD(7) remains at the end.
        # End-game BD groups rotate over FOUR distinct psum banks (psq,
        # psb-buf0 [free after tile 6's eviction], psa [free after
        # post_a(7)], psb-buf1 [free after tile 7's eviction]) so no group
        # ever waits on another's eviction round-trip.
        b_slots = {1: (NT - 3, 3, None, "psa"),
                   5: (NT - 2, 0, psb_pool, "psb"),
                   10: (NT - 2, 1, None, "psa")}
        for kk in range(KT):
            proj_chunk(p, ps_a, ps_b, kk, parts="b")
            if kk in b_slots:
                bp, gq, pool, tg = b_slots[kk]
                bd_group(bp, gq, pool=pool, tag=tg)
        post_b(p, ps_b, fast_tail=True, do_lv=False)
        # BD(6) groups 2-3 are long since ready: they fill the PE while
        # tile 7's bounce chain completes, instead of stalling the B_q pass.
        bd_group(NT - 2, 2, pool=psb_pool, tag="psb")
        bd_group(NT - 2, 3, pool=psa_pool)
        tail_pools = [(None, "psa"), (psb_pool, "psb"),
                      (psa_pool, "psa"), (None, "psa")]
        for i in range(4):
            pool, tg = tail_pools[i]
            bd_group(NT - 1, i, pool=pool, tag=tg)


def build_program():
    import concourse.tile as tile

    nc, tensors = make_nc()
    with tile.TileContext(nc) as tc:
        build_body(nc, tc, tensors)
    nc.compile()
    return nc


def _get_program():
    if "nc" not in _CACHE:
        _CACHE["nc"] = build_program()
    return _CACHE["nc"]


def make_in_maps(x, W_A_q, W_B_q, W_A_k, W_B_k, W_A_v, W_B_v):
    """Shard + preprocess full inputs into per-core input maps."""
    x = np.asarray(x)
    B, S, Hh = x.shape
    x2 = np.ascontiguousarray(x.reshape(B * S, Hh))

    # fold the 1/RQ scale and the (h,r)->(r,h) column reorder into W_A_q
    WAq = np.asarray(W_A_q).reshape(Hh, NH, RQ).transpose(0, 2, 1).reshape(
        Hh, NH * RQ
    ) / np.float32(RQ)
    Wall = np.concatenate(
        [
            WAq,
            np.asarray(W_A_k),
            np.asarray(W_A_v),
            np.asarray(W_B_k),
            np.asarray(W_B_v),
            np.asarray(W_B_q),
        ],
        axis=1,
    )
    assert Wall.shape == (Hh, NOUT)
    Wt = np.ascontiguousarray(Wall.reshape(KT, 128, NOUT)).astype(BF16)

    inv = 1.0 / (10000.0 ** (np.arange(0, HD, 2, dtype=np.float32) / HD))
    ang = np.arange(S, dtype=np.float32)[:, None] * inv[None, :]
    cs_rep = np.concatenate([np.cos(ang), np.sin(ang)], axis=1).astype(BF16)

    in_maps = []
    for i in range(8):
        tok0 = i * SH
        pos = np.arange(tok0, tok0 + SH) % S
        xt = np.ascontiguousarray(x2[tok0 : tok0 + SH].T).astype(BF16)
        x1 = xt[:, 0:256].reshape(KT, 128, 256)
        wx = np.ascontiguousarray(np.concatenate([Wt, x1], axis=2))
        # tiles 2-7 pre-tiled: [p, partition, k*128+t]
        xtp = (xt[:, 256:].reshape(KT, 128, NT - 2, 128)
               .transpose(2, 1, 0, 3).reshape(NT - 2, 128, KT * 128))
        in_maps.append(
            {
                # pre-transposed (hidden, tokens) so on-chip loads are plain
                "x": np.ascontiguousarray(xtp),
                "w": wx,
                "cs": np.ascontiguousarray(cs_rep[pos]),
            }
        )
    return in_maps, (B, S)


def assemble_outputs(results, B, S):
    # q arrives in raw block-diagonal layout: [p, t*16+h, g*128+d] with
    # token = p*128 + g*8 + t
    qs = []
    for i in range(8):
        a = results[i]["q"].astype(np.float32).reshape(NT, 8, 16, 16, 128)
        qs.append(a.transpose(0, 3, 1, 2, 4).reshape(SH, NH, HD))
    q = np.concatenate(qs, axis=0).reshape(B, S, NH, HD)
    kv = np.concatenate(
        [results[i]["kv"].astype(np.float32) for i in range(8)], axis=0
    )
    k = kv[:, 0:2048].reshape(B, S, NH, HD)
    v = kv[:, 2048:4096].reshape(B, S, NH, HD)
    return q, k, v


def kernel(x, W_A_q, W_B_q, W_A_k, W_B_k, W_A_v, W_B_v):
    from concourse.bass_utils import run_bass_kernel_spmd

    nc = _get_program()
    in_maps, (B, S) = make_in_maps(x, W_A_q, W_B_q, W_A_k, W_B_k, W_A_v, W_B_v)
    res = run_bass_kernel_spmd(nc, in_maps, list(range(8))).results
    return assemble_outputs(res, B, S)



# revision 63
# speedup vs baseline: 1.0128x; 1.0128x over previous
"""Trainium2 Bass kernel for nn_CPLinear (CP-decomposed QKV projection with RoPE).

Computes, for x:(2,4096,2048) and CP-factor weights:
    A_t = x @ W_A_t  (per-token head coefficients),  B_t = x @ W_B_t (shared bases)
    q = einsum('bshr,bsrd->bshd', A_q, rope(B_q)) / 12
    k = A_k * rope(B_k)   (rank-1)
    v = A_v * B_v         (rank-1)

Strategy (8 cores, data-parallel over the 8192 tokens, 1024 tokens/core);
HW exec ~159us mean (158-161 across runs) vs the 194us naive baseline:
  - All 6 projections fused into one [2048 x 2016] bf16 matmul; each k-chunk
    runs 4 matmuls off one stationary load (LDWEIGHTS fully amortized).
  - W streams in k-chunks, ONE whole-chunk DMA per chunk alternating the two
    HWDGE queues (halves trigger count + shared-HWDGE generation time), with
    the first two tiles' x sliver appended to W host-side; those two tiles
    run k-major, paced by chunk arrival. Remaining x arrives per-tile behind
    the stream, pre-tiled host-side into contiguous-run DMAs.
  - PE pre-warm: ~30 dummy matmuls into a scratch PSUM bank while the first
    chunk streams, so real matmuls start with the p-state ramp (0.65 ->
    1.2 -> 2.4GHz over ~3us of continuous work) already absorbed.
  - PSUM: psa[512]x1 + psb[1536]x2 + psq[512]x1 = 8 banks; stage-1 evicts
    psa(1) (parked in the psq bank) first so the BD(0) weave isn't blocked.
  - The rank-12 q contraction runs as block-diagonal matmuls (8 tokens per
    matmul, K=96, operands built via a DRAM bounce + 3-dim scatter APs;
    partition-crossing scatters cannot go SBUF->SBUF: the verifier requires
    partition-dim-first APs on the SBUF side). BD(p-2) groups 1-3 weave into
    proj(p)'s k-chunks (psq needs >=1.3us between groups for the ACT evict
    round-trip); BD(p-1) group 0 runs right after post_b(p).
  - Last tile: A-block columns first (short A'/k/v chains start early), then
    B_q; BD(5) groups weave into the A-pass, BD(6) into the B-pass, so only
    BD(7) remains after the bounce round-trip, evictions ping-ponging the
    psq/psa banks. Tail tiles 6,7 invert the bounce (8 scatter WRITES into a
    BD-layout DRAM buffer + one contiguous 4KB-row read) so bdr lands in one
    semaphore instead of eight.
  - kv ships as one combined [128,4096] DMA per tile; cos/sin tables arrive
    as a single merged SWDGE DMA; q is written in raw block-diagonal layout
    and untangled on the host.
"""

import sys

for _p in ("/opt/trn_rl_repo",):
    if _p not in sys.path:
        sys.path.insert(0, _p)

import numpy as np
import ml_dtypes

BF16 = ml_dtypes.bfloat16

SH = 1024          # tokens per core
H = 2048           # hidden
KT = H // 128      # 16 k-tiles
NT = SH // 128     # 8 token tiles per core
NOUT = 2016        # fused projection output width
WX = NOUT + 256    # W chunk + stage-1 x sliver, merged for one-DMA-per-queue
WSPL = 1136        # k-chunk DMA column split between the two HWDGE queues
NH, HD, RQ = 16, 128, 12
N_WARM = 40        # PE pre-warm dummy matmuls (128 cols each)

_CACHE = {}


def make_nc():
    import concourse.bacc as bacc
    from concourse import mybir

    dt = mybir.dt

    nc = bacc.Bacc(
        "TRN2",
        target_bir_lowering=False,
        debug=False,
        enable_asserts=False,
        num_devices=8,
    )

    x_d = nc.dram_tensor("x", (NT - 2, 128, KT * 128), dt.bfloat16, kind="ExternalInput")  # pre-tiled host-side
    w_d = nc.dram_tensor("w", (KT, 128, WX), dt.bfloat16, kind="ExternalInput")
    cs_d = nc.dram_tensor("cs", (SH, 128), dt.bfloat16, kind="ExternalInput")
    q_d = nc.dram_tensor("q", (NT, 128, NH * HD), dt.bfloat16, kind="ExternalOutput")
    kv_d = nc.dram_tensor("kv", (SH, 2 * NH * HD), dt.bfloat16, kind="ExternalOutput")
    return nc, (x_d, w_d, cs_d, q_d, kv_d)


def build_body(nc, tc, tensors):
    from contextlib import ExitStack

    from concourse import mybir

    dt = mybir.dt
    x_d, w_d, cs_d, q_d, kv_d = tensors

    with ExitStack() as ctx:
        P = ctx.enter_context
        const_pool = P(tc.tile_pool(name="const", bufs=1))
        w_sb = const_pool.tile([128, KT * WX], dt.bfloat16, tag="w_sb")
        xT = const_pool.tile([128, (NT - 2) * KT * 128], dt.bfloat16, tag="xT")
        cs_sb = const_pool.tile([128, NT * 128], dt.bfloat16, tag="cs_sb")
        warm = const_pool.tile([128, 128], dt.bfloat16, tag="warm")

        lhs_bufs = [
            const_pool.tile([128, 2048], dt.bfloat16, tag=f"lhs{i}", name=f"lhs{i}")
            for i in range(4)
        ]
        bdr_bufs = [
            const_pool.tile([128, 2048], dt.bfloat16, tag=f"bdr{i}", name=f"bdr{i}")
            for i in range(4)
        ]

        # ---- startup DMAs: k-chunked, split across both HWDGE queues ----
        # cos||sin for all 8 tiles in one SWDGE DMA (per tile p: cols
        # [p*128, p*128+64) = cos, [p*128+64, (p+1)*128) = sin)
        nc.gpsimd.dma_start(
            out=cs_sb[:].rearrange("p (t n) -> p t n", t=NT),
            in_=cs_d[:].rearrange("(t p) n -> p t n", p=128),
        )
        for tl in lhs_bufs:
            nc.gpsimd.memset(tl[:], 0.0)
        # stage-1 stream: W with tiles 0-1's x sliver appended host-side.
        # One whole-chunk DMA per k-chunk, alternating queues: halves the
        # trigger count and the shared-HWDGE generation serialization, so
        # early chunks land sooner.
        # chunks 0-2 gate the PE start: split each across both queues so
        # their transfers halve; later chunks ship whole, alternating queues
        for kk in range(3):
            nc.scalar.dma_start(
                out=w_sb[:, kk * WX : kk * WX + WSPL],
                in_=w_d[kk][:, 0:WSPL],
            )
            nc.sync.dma_start(
                out=w_sb[:, kk * WX + WSPL : (kk + 1) * WX],
                in_=w_d[kk][:, WSPL:WX],
            )
        for kk in range(3, KT):
            eng = nc.scalar if kk % 2 == 0 else nc.sync
            eng.dma_start(
                out=w_sb[:, kk * WX : (kk + 1) * WX],
                in_=w_d[kk][:, 0:WX],
            )
        # the remaining x tokens stream per-tile behind the stage-1 stream,
        # pre-tiled host-side so each per-tile load is one contiguous-run DMA
        for p in range(2, NT):
            eng = nc.sync if p % 2 == 0 else nc.scalar
            eng.dma_start(
                out=xT[:, (p - 2) * KT * 128 : (p - 1) * KT * 128],
                in_=x_d[p - 2],
            )



        psa_pool = P(tc.tile_pool(name="psa", bufs=1, space="PSUM"))
        psb_pool = P(tc.tile_pool(name="psb", bufs=2, space="PSUM"))
        psq_pool = P(tc.tile_pool(name="psq", bufs=1, space="PSUM"))
        small_pool = P(tc.tile_pool(name="small", bufs=3))
        bq_pool = P(tc.tile_pool(name="bq", bufs=2))
        bqr_pool = P(tc.tile_pool(name="bqr", bufs=2))
        rope_pool = P(tc.tile_pool(name="rope", bufs=3))
        kv_pool = P(tc.tile_pool(name="kvouts", bufs=4))
        q_pool = P(tc.tile_pool(name="qouts", bufs=4))
        scr_pool = P(tc.tile_pool(name="scr", bufs=3, space="DRAM"))

        # ---- PE pre-warm: dummy matmuls while the first W chunk streams ----
        # The PE p-state ramps 0.65->1.2->2.4GHz over ~3us of continuous
        # work; idle-starting on chunk 0 would run the first ~3us of real
        # matmuls at half speed.  Warm the array on throwaway data so the
        # first real matmul issues into a full-speed pipeline.
        nc.vector.memset(warm[:], 0.0)
        ps_warm = psq_pool.tile([128, 128], dt.float32, tag="qp", name="warm")
        for _ in range(N_WARM):
            nc.tensor.matmul(ps_warm[:], warm[:], warm[:], start=True, stop=True)

        def proj_chunk(p, ps_a, ps_b, kk, parts="ab"):
            t0 = p * 128
            if p < 2:
                lh = w_sb[:, kk * WX + NOUT + t0 : kk * WX + NOUT + t0 + 128]
            else:
                lh = xT[:, ((p - 2) * KT + kk) * 128 : ((p - 2) * KT + kk) * 128 + 128]
            wb = kk * WX
            st = kk == 0
            sp = kk == KT - 1
            if "a" in parts:
                nc.tensor.matmul(
                    ps_a[:, 0:480], lh, w_sb[:, wb : wb + 480], start=st,
                    stop=sp,
                )
            if "b" in parts:
                for c in range(3):
                    nc.tensor.matmul(
                        ps_b[:, c * 512 : (c + 1) * 512],
                        lh,
                        w_sb[:, wb + 480 + c * 512 : wb + 480 + (c + 1) * 512],
                        start=st,
                        stop=sp,
                    )

        state = {}

        def ensure_state(p):
            if p not in state:
                scr = scr_pool.tile([128, 1728], dt.bfloat16, tag="scr",
                                    name=f"scr{p}")
                state[p] = {"scr": scr}
            return state[p]

        def post_a(p, ps_a):
            """psa eviction + A' bounce + ropeK + k/v for proj tile p."""
            t0 = p * 128
            st = ensure_state(p)
            scr = st["scr"]
            smalls = small_pool.tile([128, 480], dt.bfloat16, tag="smalls")
            bkr = small_pool.tile([128, 128], dt.bfloat16, tag="bkr")
            tka = small_pool.tile([128, 64], dt.bfloat16, tag="tka")
            tkb = small_pool.tile([128, 64], dt.bfloat16, tag="tkb")
            nc.scalar.copy(smalls[:], ps_a[:, 0:480])
            # A' -> scratch (read back by l_v)
            nc.gpsimd.dma_start(out=scr[:, 1536:1728], in_=smalls[:, 0:192])
            st.update({"smalls": smalls, "bkr": bkr, "tka": tka, "tkb": tkb})

        def post_kv(p, tail=False):
            """ropeK + k/v rank-1 broadcasts + outputs for proj tile p.

            tail=True moves the v broadcast to gpsimd so the final tile's
            k and v expansions run on two engines in parallel."""
            t0 = p * 128
            st = state[p]
            smalls, bkr, tka, tkb = (st["smalls"], st["bkr"], st["tka"],
                                     st["tkb"])
            cos_k = cs_sb[:, p * 128 : p * 128 + 64]
            sin_k = cs_sb[:, p * 128 + 64 : (p + 1) * 128]
            bkv = smalls[:, 224:352].rearrange("p (two d) -> p two d", two=2)
            bkrv = bkr[:].rearrange("p (two d) -> p two d", two=2)
            nc.vector.tensor_mul(tka[:], bkv[:, 0], cos_k)
            nc.vector.tensor_mul(tkb[:], bkv[:, 1], sin_k)
            nc.vector.tensor_sub(bkrv[:, 0], tka[:], tkb[:])
            nc.vector.tensor_mul(tka[:], bkv[:, 1], cos_k)
            nc.vector.tensor_mul(tkb[:], bkv[:, 0], sin_k)
            nc.vector.tensor_add(bkrv[:, 1], tka[:], tkb[:])
            kvsb = kv_pool.tile([128, 4096], dt.bfloat16, tag="ksb")
            v_eng = nc.gpsimd if tail else nc.vector
            v_eng.tensor_mul(
                kvsb[:, 2048:4096].rearrange("p (h d) -> p h d", h=NH),
                smalls[:, 352:480].unsqueeze(1).broadcast_to([128, NH, 128]),
                smalls[:, 208:224].unsqueeze(2).broadcast_to([128, NH, 128]),
            )
            nc.vector.tensor_mul(
                kvsb[:, 0:2048].rearrange("p (h d) -> p h d", h=NH),
                bkr[:].unsqueeze(1).broadcast_to([128, NH, 128]),
                smalls[:, 192:208].unsqueeze(2).broadcast_to([128, NH, 128]),
            )
            nc.sync.dma_start(out=kv_d[t0 : t0 + 128, :], in_=kvsb[:])

        def post_b(p, ps_b, fast_tail=False, do_lv=True):
            """psb eviction, rope on B_q, bounce + scatter reads for tile p."""
            st = ensure_state(p)
            scr = st["scr"]
            bq = bq_pool.tile([128, 1536], dt.bfloat16, tag="bq")
            # last tile: evict on DVE so the ACT queue stays clear for the
            # woven BD-group evictions that pace the tail
            if fast_tail:
                nc.vector.tensor_copy(bq[:], ps_b[:])
            else:
                nc.scalar.copy(bq[:], ps_b[:])
            bqr = bqr_pool.tile([128, 1536], dt.bfloat16, tag="bqr")
            ta = rope_pool.tile([128, 768], dt.bfloat16, tag="ta")
            tb = rope_pool.tile([128, 768], dt.bfloat16, tag="tb")
            cosr = (
                cs_sb[:, p * 128 : p * 128 + 64]
                .unsqueeze(1)
                .broadcast_to([128, RQ, 64])
            )
            sinr = (
                cs_sb[:, p * 128 + 64 : (p + 1) * 128]
                .unsqueeze(1)
                .broadcast_to([128, RQ, 64])
            )
            sv = bq[:].rearrange("p (r two d) -> p r two d", r=RQ, two=2)
            dv = bqr[:].rearrange("p (r two d) -> p r two d", r=RQ, two=2)
            tav = ta[:].rearrange("p (r d) -> p r d", r=RQ)
            tbv = tb[:].rearrange("p (r d) -> p r d", r=RQ)
            p_lo = sv[:, :, 0]
            p_hi = sv[:, :, 1]
            nc.vector.tensor_mul(tav, p_lo, cosr)
            nc.vector.tensor_mul(tbv, p_hi, sinr)
            nc.vector.tensor_sub(dv[:, :, 0], tav, tbv)
            nc.vector.tensor_mul(tav, p_hi, cosr)
            nc.vector.tensor_mul(tbv, p_lo, sinr)
            nc.vector.tensor_add(dv[:, :, 1], tav, tbv)

            bdr = bdr_bufs[p % 4]
            if p >= NT - 2:
                # tail tiles: invert the bounce — 8 scatter WRITES land the
                # DRAM buffer already in BD layout, then one contiguous
                # 4KB-row read delivers all of bdr at once, so the final BD
                # groups are not paced by 8 trickling scatter-read sems.
                # The writes go out BEFORE the lv_reads so the gpsimd queue
                # doesn't head-of-line block the B-side chain.
                scr2 = scr_pool.tile([96, 2048], dt.bfloat16, tag="scr2",
                                     name=f"scr2_{p}")
                w_v = bqr[:].rearrange("(g t) (r d) -> t g r d", t=8, r=RQ)
                s_v = scr2[:].rearrange("(t r) (g d) -> t g r d", t=8, g=16)
                for t in range(8):
                    eng = (nc.sync, nc.scalar, nc.gpsimd)[t % 3]
                    eng.dma_start(out=s_v[t], in_=w_v[t])
                nc.sync.dma_start(out=bdr[0:96, :], in_=scr2[:])
                st["bdr"] = bdr
                if do_lv:
                    lv_reads(p)
                return
            # bounce roped B_q, then scatter-read the block-diagonal operands
            nc.scalar.dma_start(out=scr[:, 0:1536], in_=bqr[:])
            sb_v = scr[:, 0:1536].rearrange("(g t) (r d) -> t r g d", t=8, r=RQ)
            d_v = bdr[0:96, :].rearrange("(t r) (g d) -> t r g d", t=8, g=16)
            for t in range(8):
                eng = (nc.sync, nc.scalar, nc.gpsimd)[t % 3]
                eng.dma_start(out=d_v[t], in_=sb_v[t])
            st["bdr"] = bdr
            if do_lv:
                lv_reads(p)

        def lv_reads(p, spread=False):
            scr = state[p]["scr"]
            lhs = lhs_bufs[p % 4]
            sa_v = scr[:, 1536:1728].rearrange(
                "(g t) (r h) -> t r g h", t=8, r=RQ
            )
            l_v = lhs[0:96, :].rearrange("(t r) (g c) -> t r g c", t=8, g=16)
            for t in range(8):
                if spread:
                    eng = (nc.scalar, nc.gpsimd)[t % 2]
                else:
                    eng = nc.gpsimd
                eng.dma_start(
                    out=l_v[t][:, :, t * 16 : (t + 1) * 16], in_=sa_v[t]
                )
            state[p]["lhs"] = lhs

        def bd_group(p, gq, pool=None, tag="psa"):
            """one PSUM-bank group (4 block-diagonal matmuls) of tile p's q."""
            st = state[p]
            if gq == 0:
                st["qsb"] = q_pool.tile([128, 2048], dt.bfloat16, tag="qsb",
                                        name=f"qsb{p}")
            lhs, bdr, qsb = st["lhs"], st["bdr"], st["qsb"]
            if pool is None:
                qp = psq_pool.tile([128, 512], dt.float32, tag="qp",
                                   name=f"qp{p}_{gq}")
            else:
                qp = pool.tile([128, 512], dt.float32, tag=tag,
                               name=f"qp{p}_{gq}")
            for j4 in range(4):
                g = gq * 4 + j4
                nc.tensor.matmul(
                    qp[:, j4 * 128 : (j4 + 1) * 128],
                    lhs[0:96, g * 128 : (g + 1) * 128],
                    bdr[0:96, g * 128 : (g + 1) * 128],
                    start=True,
                    stop=True,
                )
            nc.scalar.copy(qsb[:, gq * 512 : (gq + 1) * 512], qp[:])
            if p == NT - 1:
                # last tile: ship each PSUM-group slice as soon as it evicts
                nc.sync.dma_start(
                    out=q_d[p][:, gq * 512 : (gq + 1) * 512],
                    in_=qsb[:, gq * 512 : (gq + 1) * 512],
                )
            elif gq == 3:
                nc.scalar.dma_start(out=q_d[p], in_=qsb[:])

        # ================= schedule =================
        # stage 1: proj tiles 0,1 k-major, paced by the chunked W/x DMAs.
        # tile 1's A-block accumulates in the (otherwise idle) psq pool.
        ps_a0 = psa_pool.tile([128, 512], dt.float32, tag="psa", name="psa0")
        ps_b0 = psb_pool.tile([128, 1536], dt.float32, tag="psb", name="psb0")
        ps_a1 = psq_pool.tile([128, 512], dt.float32, tag="qp", name="psa1")
        ps_b1 = psb_pool.tile([128, 1536], dt.float32, tag="psb", name="psb1")
        for kk in range(KT):
            proj_chunk(0, ps_a0, ps_b0, kk)
            proj_chunk(1, ps_a1, ps_b1, kk)
        post_a(0, ps_a0)
        post_a(1, ps_a1)  # frees the psq bank early for the BD(0) weave
        post_b(0, ps_b0)
        post_b(1, ps_b1)
        post_kv(0)
        post_kv(1)

        # stage 2: tiles 2-6 tile-major with the BD contraction woven in.
        # proj(3) carries the BD(0)/BD(1) catch-up; from then on BD(p-1)
        # starts at post(p) and BD(p-2) groups 1-3 run inside proj(p).
        weave = {
            3: [(0, 0, 1), (0, 1, 3), (0, 2, 5), (0, 3, 7),
                (1, 0, 9), (1, 1, 11), (1, 2, 13), (1, 3, 15)],
        }
        for p in range(4, NT):
            weave[p] = [(p - 2, 1, 3), (p - 2, 2, 7), (p - 2, 3, 11)]
        for p in range(2, NT - 1):
            ps_a = psa_pool.tile([128, 512], dt.float32, tag="psa",
                                 name=f"psa{p}")
            ps_b = psb_pool.tile([128, 1536], dt.float32, tag="psb",
                                 name=f"psb{p}")
            slots = {kk: (bp, gq) for (bp, gq, kk) in weave.get(p, [])}
            for kk in range(KT):
                proj_chunk(p, ps_a, ps_b, kk)
                if kk in slots:
                    bd_group(*slots[kk])
            post_a(p, ps_a)
            post_b(p, ps_b)
            if p >= 3:
                bd_group(p - 1, 0)
            post_kv(p)
        # last tile: A-block columns first so the A'/k/v chains run during
        # the B_q pass, which in turn ends early enough that the bounce
        # round-trip hides under the reserved BD groups
        p = NT - 1
        ps_a = psa_pool.tile([128, 512], dt.float32, tag="psa", name="psa7")
        ps_b = psb_pool.tile([128, 1536], dt.float32, tag="psb", name="psb7")
        a_slots = {4: (NT - 3, 1), 12: (NT - 3, 2)}
        for kk in range(KT):
            proj_chunk(p, ps_a, ps_b, kk, parts="a")
            if kk in a_slots:
                bd_group(*a_slots[kk])
        post_a(p, ps_a)
        lv_reads(p)
        post_kv(p)
        # BD(5)/BD(6) finish inside the B_q pass, so their evictions and q
        # DMAs land before the tail crunch; only BD(7) remains at the end.
        b_slots = {1: (NT - 3, 3), 5: (NT - 2, 0), 10: (NT - 2, 1)}
        for kk in range(KT):
            proj_chunk(p, ps_a, ps_b, kk, parts="b")
            if kk in b_slots:
                bd_group(*b_slots[kk])
        post_b(p, ps_b, fast_tail=True, do_lv=False)
        # BD(6) groups 2-3 are long since ready: they fill the PE while
        # tile 7's bounce chain completes, instead of stalling the B_q pass
        # on the single psq bank.
        bd_group(NT - 2, 2)
        bd_group(NT - 2, 3, pool=psa_pool)
        for i in range(4):
            bd_group(NT - 1, i, pool=(psa_pool if i % 2 == 1 else None))


def build_program():
    import concourse.tile as tile

    nc, tensors = make_nc()
    with tile.TileContext(nc) as tc:
        build_body(nc, tc, tensors)
    nc.compile()
    return nc


def _get_program():
    if "nc" not in _CACHE:
        _CACHE["nc"] = build_program()
    return _CACHE["nc"]


def make_in_maps(x, W_A_q, W_B_q, W_A_k, W_B_k, W_A_v, W_B_v):
    """Shard + preprocess full inputs into per-core input maps."""
    x = np.asarray(x)
    B, S, Hh = x.shape
    x2 = np.ascontiguousarray(x.reshape(B * S, Hh))

    # fold the 1/RQ scale and the (h,r)->(r,h) column reorder into W_A_q
    WAq = np.asarray(W_A_q).reshape(Hh, NH, RQ).transpose(0, 2, 1).reshape(
        Hh, NH * RQ
    ) / np.float32(RQ)
    Wall = np.concatenate(
        [
            WAq,
            np.asarray(W_A_k),
            np.asarray(W_A_v),
            np.asarray(W_B_k),
            np.asarray(W_B_v),
            np.asarray(W_B_q),
        ],
        axis=1,
    )
    assert Wall.shape == (Hh, NOUT)
    Wt = np.ascontiguousarray(Wall.reshape(KT, 128, NOUT)).astype(BF16)

    inv = 1.0 / (10000.0 ** (np.arange(0, HD, 2, dtype=np.float32) / HD))
    ang = np.arange(S, dtype=np.float32)[:, None] * inv[None, :]
    cs_rep = np.concatenate([np.cos(ang), np.sin(ang)], axis=1).astype(BF16)

    in_maps = []
    for i in range(8):
        tok0 = i * SH
        pos = np.arange(tok0, tok0 + SH) % S
        xt = np.ascontiguousarray(x2[tok0 : tok0 + SH].T).astype(BF16)
        x1 = xt[:, 0:256].reshape(KT, 128, 256)
        wx = np.ascontiguousarray(np.concatenate([Wt, x1], axis=2))
        # tiles 2-7 pre-tiled: [p, partition, k*128+t]
        xtp = (xt[:, 256:].reshape(KT, 128, NT - 2, 128)
               .transpose(2, 1, 0, 3).reshape(NT - 2, 128, KT * 128))
        in_maps.append(
            {
                # pre-transposed (hidden, tokens) so on-chip loads are plain
                "x": np.ascontiguousarray(xtp),
                "w": wx,
                "cs": np.ascontiguousarray(cs_rep[pos]),
            }
        )
    return in_maps, (B, S)


def assemble_outputs(results, B, S):
    # q arrives in raw block-diagonal layout: [p, t*16+h, g*128+d] with
    # token = p*128 + g*8 + t
    qs = []
    for i in range(8):
        a = results[i]["q"].astype(np.float32).reshape(NT, 8, 16, 16, 128)
        qs.append(a.transpose(0, 3, 1, 2, 4).reshape(SH, NH, HD))
    q = np.concatenate(qs, axis=0).reshape(B, S, NH, HD)
    kv = np.concatenate(
        [results[i]["kv"].astype(np.float32) for i in range(8)], axis=0
    )
    k = kv[:, 0:2048].reshape(B, S, NH, HD)
    v = kv[:, 2048:4096].reshape(B, S, NH, HD)
    return q, k, v


def kernel(x, W_A_q, W_B_q, W_A_k, W_B_k, W_A_v, W_B_v):
    from concourse.bass_utils import run_bass_kernel_spmd

    nc = _get_program()
    in_maps, (B, S) = make_in_maps(x, W_A_q, W_B_q, W_A_k, W_B_k, W_A_v, W_B_v)
    res = run_bass_kernel_spmd(nc, in_maps, list(range(8))).results
    return assemble_outputs(res, B, S)

